# revision 14
# baseline (speedup 1.0000x reference)
"""Trainium2 Bass kernel for DynamicFilterWithImageInput.

Model (per batch b):
  img_feat = mean_hw(relu(BN1(conv2d(raw_img, w_conv1, 3x3, zeropad=1) + b1)))   # (64,)
  df       = softmax_over_C(BN2(img_feat @ w_filt.T + b_filt).reshape(C, K*K))   # (C, 25)
  out      = depthwise_conv5x5(reflect_pad(x_feat), df)                          # (C, H, W)

Sharding: pure data-parallel over batch (16 batches -> 8 cores x 2 batches).

Device mapping (per core, B_PC=2 batches), v3:
  - Head: conv1 as K=54 matmuls over 512-col chunks (4-deep PSUM pipe),
    ReLU+bias+spatial-sum alternating ScalarE/VectorE via accum_out; dense
    (K=65, bias row folded); softmax on [50,256]; filter transpose via PE.
    Consts + x slabs ride the ACT DMA queue so the conv1 im2col stream on
    the sync queue is never blocked.
  - Depthwise 5x5 per slab (b, channel-group-of-128), per quad (16 rows):
    n_pe taps on PE (diag-weight bf16 matmuls, PSUM accumulate), n_dve taps
    on VectorE as scalar_tensor_tensor chains accumulating straight into
    PSUM (PSUM reads bypass SBUF ports -> no DVE/GpSimd shared-port traffic,
    no fold passes), optional n_act taps on ScalarE folded in by VectorE.
    ScalarE evacuates PSUM (fp32->bf16) and the store DMA runs one quad
    behind compute.  GpSimd is deliberately idle: any GpSimd op would lock
    the SBUF port pair it shares with VectorE and stall the tap chains.
"""

import os
import sys

sys.path.insert(0, "/opt/trn_rl_repo")

import numpy as np
import ml_dtypes

import concourse.bass as bass
import concourse.bacc as bacc
import concourse.mybir as mybir
import concourse.tile as tile
from concourse.bass_utils import run_bass_kernel_spmd
import concourse.dve_ops as _dve_ops


def _get_pair_mac():
    """Fused custom DVE op: out = in0*s0 + in1*s1 (two conv taps per pass).
    Registered dynamically so kernel.py is self-contained."""
    if hasattr(_dve_ops, "PAIR_MAC_ANT"):
        return _dve_ops.PAIR_MAC_ANT
    from concourse.dve_spec import Spec, Src0, C0, C1
    from concourse.dve_spec import Src1
    op = _dve_ops.DveOp(
        "PAIR_MAC_ANT",
        Spec(
            body=Src0 * C0 + Src1 * C1,
            reference=lambda in0, in1, s0, s1, imm2: (
                in0.astype(np.float32) * s0 + in1.astype(np.float32) * s1
            ).astype(np.float32),
        ),
        subdim=False,
        uops_sha={"v3": "f2ac165a27dbafb3", "v4": "49eb47656a95aba3"},
    )
    _dve_ops.OPS.append(op)
    _dve_ops.CUSTOM_DVE_SPECS[op.name] = op.spec
    _dve_ops._SUB_OPCODE_FOR_NAME[op.name] = (
        _dve_ops._CUSTOM_DVE_ROW_BASE + len(_dve_ops.OPS) - 1
    )
    assert max(_dve_ops._SUB_OPCODE_FOR_NAME.values()) < 0x20
    _dve_ops.PAIR_MAC_ANT = op
    return op


PAIR_MAC_ANT = _get_pair_mac()

BF16 = mybir.dt.bfloat16
F32 = mybir.dt.float32
AF = mybir.ActivationFunctionType
ALU = mybir.AluOpType

EPS = 1e-5
B_PC = 2          # batches per core
C = 256           # channels
CG = C // 128     # channel groups of 128
K5 = 5            # depthwise kernel size
TAPS = [(i, j) for i in range(K5) for j in range(K5)]
NSLAB = B_PC * CG

_PROG_CACHE = {}


def _build_program(H, W, n_dve=8, n_act=0):
    """Per-core Tile program.  Per quad: n_act taps on ScalarE, n_dve on
    VectorE (STT chains into PSUM), rest on PE."""
    Hp, Wp = H + 4, W + 4
    HWOUT = H * W
    GR = min(H, max(1, 512 // W))        # rows per matmul group (1 psum bank)
    QG = 4                               # matmul groups per quad (psum banks)
    QR = GR * QG                         # output rows per quad
    if H % QR != 0:
        QG = 1
        QR = GR
    assert H % QR == 0
    NQ = H // QR
    N1CH = min(512, HWOUT)               # conv1 psum chunk
    assert HWOUT % N1CH == 0
    N1 = HWOUT // N1CH                   # number of conv1 chunks
    IMCH = min(4 * N1CH, HWOUT)          # im2col streaming chunk

    assert n_dve + n_act <= 25

    nc = bacc.Bacc("TRN2", target_bir_lowering=False, debug=False)

    x_d = nc.dram_tensor("x", [B_PC, C, Hp, Wp], BF16, kind="ExternalInput").ap()
    im2col_d = nc.dram_tensor("im2col", [54, HWOUT], BF16, kind="ExternalInput").ap()
    wconv_d = nc.dram_tensor("wconv", [54, 128], BF16, kind="ExternalInput").ap()
    b1r_d = nc.dram_tensor("b1r", [128, 1], F32, kind="ExternalInput").ap()
    wft_d = nc.dram_tensor("wft", [65, 25, C], BF16, kind="ExternalInput").ap()
    ident_d = nc.dram_tensor("ident", [128, 128], BF16, kind="ExternalInput").ap()
    id32_d = nc.dram_tensor("id32", [128, 128], F32, kind="ExternalInput").ap()
    out_d = nc.dram_tensor("out", [B_PC, C, H, W], BF16, kind="ExternalOutput").ap()

    # dram scratch for layout bounces
    imgf_d = nc.dram_tensor("imgf_sc", [128], F32).ap()
    df_d = nc.dram_tensor("df_sc", [B_PC, 25, C], F32).ap()

    with tile.TileContext(nc) as tc:
        with (
            tc.tile_pool(name="consts", bufs=1) as consts,
            tc.tile_pool(name="p0", bufs=1) as p0,
            tc.tile_pool(name="imc", bufs=4) as imcp,
            tc.tile_pool(name="trash", bufs=2) as trashp,
            tc.tile_pool(name="xp", bufs=2) as xpp,
            # distinct diag tiles for all 4 slabs: a ring that aliases across
            # slabs would head-of-line-block the generating engine's queue
            tc.tile_pool(name="diag", bufs=NSLAB * 25) as diagp,
            tc.tile_pool(name="qt", bufs=max(n_dve // 2, 1) + 1) as qp,
            tc.tile_pool(name="at", bufs=max(n_act, 1) + 1) as atp,
            tc.tile_pool(name="et", bufs=3) as etp,
        ):
            # ---------- DMA: sync queue feeds conv1; ACT queue gets the rest
            wconv = consts.tile([54, 128], BF16)
            b1r = consts.tile([128, 1], F32)
            wft = consts.tile([65, 25, C], BF16)
            ident = consts.tile([128, 128], BF16)
            id32 = consts.tile([128, 128], F32)
            zeros = consts.tile([128, N1CH], F32)
            nc.sync.dma_start(wconv[:], wconv_d[:])
            nc.sync.dma_start(b1r[:], b1r_d[:])
            nc.scalar.dma_start(wft[:], wft_d[:])
            nc.scalar.dma_start(ident[:], ident_d[:])
            nc.scalar.dma_start(id32[:], id32_d[:])
            nc.vector.memset(zeros[:], 0.0)

            # whole im2col prefetched before the big x-slab loads so the
            # conv1 stream never starves behind them
            imts = []
            for o0 in range(0, HWOUT, IMCH):
                imt = imcp.tile([54, IMCH], BF16, tag="imc")
                nc.scalar.dma_start(imt[:], im2col_d[:, o0:o0 + IMCH])
                imts.append(imt)

            xps = [None] * NSLAB

            def load_slab(s):
                # flat alloc with 8 spare elements so full-width runs for the
                # bottom-most taps stay in-bounds
                b, cg = divmod(s, CG)
                xpf = xpp.tile([128, Hp * Wp + 8], BF16, tag="xp")
                nc.vector.memset(xpf[:, Hp * Wp:], 0.0)
                xp3 = xpf[:, 0:Hp * Wp].rearrange("p (a b) -> p a b", a=Hp, b=Wp)
                nc.scalar.dma_start(xp3, x_d[b, cg * 128:(cg + 1) * 128, :, :])
                xps[s] = (xpf, xp3)

            load_slab(0)
            load_slab(1)

            # ---------- head: conv1 + dense + softmax + transposes ----------
            acc = p0.tile([128, N1], F32)
            vts = []
            with tc.tile_pool(name="psA", bufs=4, space="PSUM") as psA:
                for ci in range(N1):
                    imt = imts[ci // (IMCH // N1CH)]
                    ps1 = psA.tile([128, N1CH], F32, tag="c")
                    off = (ci % (IMCH // N1CH)) * N1CH
                    nc.tensor.matmul(
                        ps1[:], wconv[:], imt[:, off:off + N1CH],
                        start=True, stop=True,
                    )
                    tr = trashp.tile([128, N1CH], BF16, tag="tr")
                    if ci % 2 == 0:
                        nc.scalar.activation(
                            tr[:], ps1[:], AF.Relu, bias=b1r[:], scale=1.0,
                            accum_out=acc[:, ci:ci + 1],
                        )
                    else:
                        nc.vector.scalar_tensor_tensor(
                            tr[:], ps1[:], b1r[:], zeros[:], ALU.add, ALU.max,
                            accum_out=acc[:, ci:ci + 1],
                        )
                sfeat = p0.tile([128, 1], F32)
                nc.vector.tensor_reduce(sfeat[:], acc[:], mybir.AxisListType.X, ALU.add)
                nc.sync.dma_start(imgf_d[:], sfeat[:])

                # img_feat (sums) transposed [64,2]; mean folded into the cast
                imgfT32 = p0.tile([65, B_PC], F32)
                nc.sync.dma_start(
                    imgfT32[0:64, :],
                    imgf_d[:].rearrange("(b o) -> o b", b=B_PC, o=64),
                )
                imgfT = p0.tile([65, B_PC], BF16)
                nc.vector.memset(imgfT[64:65, :], 1.0)
                nc.scalar.mul(imgfT[0:64, :], imgfT32[0:64, :], 1.0 / HWOUT)

                # dense: df[b, t, c] (+bias row); bounce PSUM->SBUF->DRAM
                t0 = 0
                di = 0
                while t0 < 25:
                    tw = min(2, 25 - t0)
                    psd = psA.tile([B_PC, 2, C], F32, tag="c")
                    nc.tensor.matmul(
                        psd[:, 0:tw, :], imgfT[:],
                        wft[:, t0:t0 + tw, :],
                        start=True, stop=True,
                    )
                    dfc = trashp.tile([B_PC, 2, C], F32, tag="dfc")
                    if di % 2 == 0:
                        nc.scalar.copy(dfc[:, 0:tw, :], psd[:, 0:tw, :])
                    else:
                        nc.vector.tensor_copy(dfc[:, 0:tw, :], psd[:, 0:tw, :])
                    nc.sync.dma_start(df_d[:, t0:t0 + tw, :], dfc[:, 0:tw, :])
                    t0 += tw
                    di += 1

                # softmax over channels; batch b parked at partition b*32 so
                # the PE transpose below sees base partitions in {0, 32}
                dfsb = p0.tile([B_PC * 32, C], F32)
                edf = p0.tile([B_PC * 32, C], F32)
                ssum = p0.tile([B_PC * 32, 1], F32)
                rsum = p0.tile([B_PC * 32, 1], F32)
                wsm = p0.tile([B_PC * 32, C], F32)
                for b in range(B_PC):
                    sl = slice(b * 32, b * 32 + 25)
                    nc.sync.dma_start(dfsb[sl, :], df_d[b])
                    nc.scalar.activation(edf[sl, :], dfsb[sl, :], AF.Exp)
                    nc.vector.tensor_reduce(
                        ssum[sl, :], edf[sl, :], mybir.AxisListType.X, ALU.add)
                    nc.vector.reciprocal(rsum[sl, :], ssum[sl, :])
                    nc.vector.tensor_scalar(
                        wsm[sl, :], edf[sl, :], rsum[sl, :], None, ALU.mult)

                # per-slab filter values [128(c), 25] via PE transpose
                for s in range(NSLAB):
                    b, cg = divmod(s, CG)
                    pst = psA.tile([128, 25], F32, tag="t")
                    nc.tensor.transpose(
                        pst[:], wsm[b * 32:b * 32 + 25, cg * 128:(cg + 1) * 128],
                        id32[b * 32:b * 32 + 25, 0:25],
                    )
                    vt = p0.tile([128, 25], F32, tag=f"vt{s}")
                    nc.scalar.copy(vt[:], pst[:])
                    vts.append(vt)

            # ---------- depthwise ----------
            # PE: n_pe taps (diag matmuls, PSUM accumulate); DVE: n_dve taps
            # as scalar_tensor_tensor chains accumulating straight into PSUM
            # (PSUM reads bypass the SBUF ports, so no shared-port traffic and
            # no fold passes); ACT: n_act taps as scale-multiplies folded into
            # the evacuated output by DVE tensor_tensor (one-quad lag).
            n_pe = 25 - n_dve - n_act
            pe_taps = TAPS[:n_pe]
            dve_taps = TAPS[n_pe:n_pe + n_dve]
            act_taps = TAPS[n_pe + n_dve:]

            dts_all = [None] * NSLAB
            _dgen_rr = [0]

            def gen_diags(s, split=False):
                # diag tiles for slab s on ScalarE (otherwise idle); for the
                # head slabs also use VectorE (idle there) so PE's first taps
                # are not production-paced.  Only PE taps need diags.
                dts = [None] * 25
                for k, (i, j) in enumerate(pe_taps):
                    t = i * K5 + j
                    dt_ = diagp.tile([128, 128], BF16, tag="dt")
                    if split and k % 2 == 0:
                        nc.vector.tensor_scalar(
                            dt_[:], ident[:], vts[s][:, t:t + 1], None, ALU.mult)
                    else:
                        nc.scalar.mul(dt_[:], ident[:], vts[s][:, t:t + 1])
                    dts[t] = dt_
                dts_all[s] = dts

            gen_diags(0, split=True)
            gen_diags(1, split=True)

            with tc.tile_pool(name="psQ", bufs=2, space="PSUM") as psQ:
                for s in range(NSLAB):
                    b, cg = divmod(s, CG)
                    vt = vts[s]
                    xpf, xp = xps[s]
                    dts = dts_all[s]
                    if s + 2 < NSLAB:
                        load_slab(s + 2)
                    if s + 1 < NSLAB and s > 0:
                        gen_diags(s + 1)

                    for q in range(NQ):
                        y0 = q * QR

                        def win(i, j):
                            return xp[:, y0 + i:y0 + i + QR, j:j + W]

                        def sc(i, j):
                            t = i * K5 + j
                            return vt[:, t:t + 1]

                        def run(i, j):
                            off = (y0 + i) * Wp + j
                            return xpf[:, off:off + QR * Wp]

                        # ACT taps -> partial tiles (optional)
                        ats = []
                        for (i, j) in act_taps:
                            at = atp.tile([128, QR * Wp], BF16, tag="at")
                            nc.scalar.mul(at[:], run(i, j), sc(i, j))
                            ats.append(at)

                        # DVE pair taps -> partial tiles (independent, so the
                        # DVE queue streams them back to back with no RAW gap)
                        qs = []
                        for pi in range(n_dve // 2):
                            ia, ja = dve_taps[2 * pi]
                            ib, jb = dve_taps[2 * pi + 1]
                            qt = qp.tile([128, QR * Wp], BF16, tag="q")
                            nc.vector._custom_dve(
                                PAIR_MAC_ANT, out=qt[:],
                                in0=run(ia, ja), in1=run(ib, jb),
                                s0=sc(ia, ja), s1=sc(ib, jb))
                            qs.append(qt)

                        # PE taps -> PSUM
                        ps = psQ.tile([128, QR, W], F32, tag="ps")
                        for g in range(QG):
                            gy = y0 + g * GR
                            for k, (i, j) in enumerate(pe_taps):
                                nc.tensor.matmul(
                                    ps[:, g * GR:(g + 1) * GR, :],
                                    dts[i * K5 + j][:],
                                    xp[:, gy + i:gy + i + GR, j:j + W],
                                    start=(k == 0),
                                    stop=(k == len(pe_taps) - 1),
                                )

                        def v(tile_):
                            return tile_[:].rearrange(
                                "p (a b) -> p a b", a=QR, b=Wp)[:, :, 0:W]

                        # fold pair partials (stock TT runs at 2x) and ACT
                        # partials into qs[0]
                        for k in range(1, len(qs)):
                            nc.vector.tensor_tensor(
                                qs[0][:], qs[0][:], qs[k][:], ALU.add)
                        for at in ats:
                            nc.vector.tensor_tensor(
                                qs[0][:], qs[0][:], at[:], ALU.add)
                        if n_dve % 2:
                            i, j = dve_taps[-1]
                            nc.vector.scalar_tensor_tensor(
                                ps[:], win(i, j), sc(i, j), ps[:],
                                ALU.mult, ALU.add)

                        # ACT evacuates PSUM the moment PE stops (DVE never
                        # touches ps with pure pair taps, so PSUM turnover is
                        # decoupled from the DVE stream); DVE folds its side in
                        # with one 2x tensor_tensor.
                        et = etp.tile([128, QR, W], BF16, tag="et")
                        nc.scalar.copy(et[:], ps[:])
                        if qs:
                            nc.vector.tensor_tensor(
                                et[:], et[:], v(qs[0]), ALU.add)
                        nc.sync.dma_start(
                            out_d[b, cg * 128:(cg + 1) * 128, y0:y0 + QR, :],
                            et[:])

    nc.compile()
    return nc


def get_program(H, W, n_dve=8, n_act=0):
    key = (H, W, n_dve, n_act)
    if key not in _PROG_CACHE:
        _PROG_CACHE[key] = _build_program(H, W, n_dve, n_act)
    return _PROG_CACHE[key]


def host_prep(x_feat, raw_img, w_conv1, b_conv1, g1, beta1, m1, v1,
              w_filt, b_filt, g2, beta2, m2, v2):
    """Fold BN params, build im2col + packed weights; returns per-core in_maps."""
    B, Cc, H, W = x_feat.shape
    assert Cc == C
    n_cores = B // B_PC

    a1 = g1 / np.sqrt(v1 + EPS)
    w1f = (w_conv1 * a1[:, None, None, None]).astype(np.float32)   # (64,3,3,3)
    b1f = (b_conv1 - m1) * a1 + beta1                               # (64,)

    a2 = g2 / np.sqrt(v2 + EPS)
    wff = (w_filt * a2[:, None]).astype(np.float32)                 # (6400,64)
    bff = (b_filt - m2) * a2 + beta2                                # (6400,)

    # wft[k, t, c]: k<64 -> wff[c*25+t, k]; k=64 -> bias row (contiguous
    # channel runs so the dense matmul's moving operand streams fast)
    wft = np.empty((65, 25, C), np.float32)
    wft[:64] = wff.reshape(C, 25, 64).transpose(2, 1, 0)
    wft[64] = bff.reshape(C, 25).T
    wft16 = wft.astype(ml_dtypes.bfloat16)

    b1r = np.tile(b1f, B_PC).reshape(128, 1).astype(np.float32)

    ident = np.eye(128, dtype=ml_dtypes.bfloat16)
    # 25x25 identity blocks at partition offsets 0 and 32 (PE-transpose
    # requires the identity operand at the same base partition as the input)
    id32 = np.zeros((128, 128), np.float32)
    for b in range(B_PC):
        id32[b * 32:b * 32 + 25, 0:25] = np.eye(25)

    xpad16 = np.pad(x_feat, ((0, 0), (0, 0), (2, 2), (2, 2)),
                    mode="reflect").astype(ml_dtypes.bfloat16)

    # conv1 im2col, zero pad 1: [54, H*W] per core
    rawpad = np.pad(raw_img, ((0, 0), (0, 0), (1, 1), (1, 1))).astype(np.float32)

    # wconv[b*27 + (c*9+i*3+j), b*64+o] = w1f[o, c, i, j]
    wconv = np.zeros((54, 128), np.float32)
    w_flat = w1f.transpose(1, 2, 3, 0).reshape(27, 64)  # (c*9+i*3+j, o)
    for b in range(B_PC):
        wconv[b * 27:(b + 1) * 27, b * 64:(b + 1) * 64] = w_flat
    wconv16 = wconv.astype(ml_dtypes.bfloat16)

    in_maps = []
    for core in range(n_cores):
        bs = core * B_PC
        im2col = np.empty((54, H * W), np.float32)
        for b in range(B_PC):
            for c in range(3):
                for i in range(3):
                    for j in range(3):
                        p = b * 27 + c * 9 + i * 3 + j
                        im2col[p] = rawpad[bs + b, c, i:i + H, j:j + W].reshape(-1)
        in_maps.append({
            "x": xpad16[bs:bs + B_PC],
            "im2col": im2col.astype(ml_dtypes.bfloat16),
            "wconv": wconv16,
            "b1r": b1r,
            "wft": wft16,
            "ident": ident,
            "id32": id32,
        })
    return in_maps


def run(inputs, trace=False, n_dve=8, n_act=0):
    x_feat = inputs["x_feat"]
    B, _, H, W = x_feat.shape
    nc = get_program(H, W, n_dve, n_act)
    in_maps = host_prep(**inputs)
    n_cores = len(in_maps)
    res = run_bass_kernel_spmd(nc, in_maps, list(range(n_cores)), trace=trace)
    out = np.concatenate(
        [r["out"].astype(np.float32) for r in res.results], axis=0)
    return out, res


def kernel(**inputs) -> np.ndarray:
    out, _ = run(inputs, trace=False)
    return out


# revision 15
# speedup vs baseline: 1.0257x; 1.0257x over previous
"""Trainium2 Bass kernel for DynamicFilterWithImageInput.

Model (per batch b):
  img_feat = mean_hw(relu(BN1(conv2d(raw_img, w_conv1, 3x3, zeropad=1) + b1)))   # (64,)
  df       = softmax_over_C(BN2(img_feat @ w_filt.T + b_filt).reshape(C, K*K))   # (C, 25)
  out      = depthwise_conv5x5(reflect_pad(x_feat), df)                          # (C, H, W)

Sharding: pure data-parallel over batch (16 batches -> 8 cores x 2 batches).

Device mapping (per core, B_PC=2 batches), v3:
  - Head: conv1 as K=54 matmuls over 512-col chunks (4-deep PSUM pipe),
    ReLU+bias+spatial-sum alternating ScalarE/VectorE via accum_out; dense
    (K=65, bias row folded); softmax on [50,256]; filter transpose via PE.
    Consts + x slabs ride the ACT DMA queue so the conv1 im2col stream on
    the sync queue is never blocked.
  - Depthwise 5x5 per slab (b, channel-group-of-128), per quad (16 rows):
    n_pe taps on PE (diag-weight bf16 matmuls, PSUM accumulate), n_dve taps
    on VectorE as scalar_tensor_tensor chains accumulating straight into
    PSUM (PSUM reads bypass SBUF ports -> no DVE/GpSimd shared-port traffic,
    no fold passes), optional n_act taps on ScalarE folded in by VectorE.
    ScalarE evacuates PSUM (fp32->bf16) and the store DMA runs one quad
    behind compute.  GpSimd is deliberately idle: any GpSimd op would lock
    the SBUF port pair it shares with VectorE and stall the tap chains.
"""

import os
import sys

sys.path.insert(0, "/opt/trn_rl_repo")

import numpy as np
import ml_dtypes

import concourse.bass as bass
import concourse.bacc as bacc
import concourse.mybir as mybir
import concourse.tile as tile
from concourse.bass_utils import run_bass_kernel_spmd
import concourse.dve_ops as _dve_ops


def _get_pair_mac():
    """Fused custom DVE op: out = in0*s0 + in1*s1 (two conv taps per pass).
    Registered dynamically so kernel.py is self-contained."""
    if hasattr(_dve_ops, "PAIR_MAC_ANT"):
        return _dve_ops.PAIR_MAC_ANT
    from concourse.dve_spec import Spec, Src0, C0, C1
    from concourse.dve_spec import Src1
    op = _dve_ops.DveOp(
        "PAIR_MAC_ANT",
        Spec(
            body=Src0 * C0 + Src1 * C1,
            reference=lambda in0, in1, s0, s1, imm2: (
                in0.astype(np.float32) * s0 + in1.astype(np.float32) * s1
            ).astype(np.float32),
        ),
        subdim=False,
        uops_sha={"v3": "f2ac165a27dbafb3", "v4": "49eb47656a95aba3"},
    )
    _dve_ops.OPS.append(op)
    _dve_ops.CUSTOM_DVE_SPECS[op.name] = op.spec
    _dve_ops._SUB_OPCODE_FOR_NAME[op.name] = (
        _dve_ops._CUSTOM_DVE_ROW_BASE + len(_dve_ops.OPS) - 1
    )
    assert max(_dve_ops._SUB_OPCODE_FOR_NAME.values()) < 0x20
    _dve_ops.PAIR_MAC_ANT = op
    return op


PAIR_MAC_ANT = _get_pair_mac()

BF16 = mybir.dt.bfloat16
F32 = mybir.dt.float32
AF = mybir.ActivationFunctionType
ALU = mybir.AluOpType

EPS = 1e-5
B_PC = 2          # batches per core
C = 256           # channels
CG = C // 128     # channel groups of 128
K5 = 5            # depthwise kernel size
TAPS = [(i, j) for i in range(K5) for j in range(K5)]
NSLAB = B_PC * CG

_PROG_CACHE = {}


def _build_program(H, W, n_dve=8, n_act=0):
    """Per-core Tile program.  Per quad: n_act taps on ScalarE, n_dve on
    VectorE (STT chains into PSUM), rest on PE."""
    Hp, Wp = H + 4, W + 4
    HWOUT = H * W
    GR = min(H, max(1, 512 // W))        # rows per matmul group (1 psum bank)
    QG = 4                               # matmul groups per quad (psum banks)
    QR = GR * QG                         # output rows per quad
    if H % QR != 0:
        QG = 1
        QR = GR
    assert H % QR == 0
    NQ = H // QR
    N1CH = min(512, HWOUT)               # conv1 psum chunk
    assert HWOUT % N1CH == 0
    N1 = HWOUT // N1CH                   # number of conv1 chunks
    IMCH = min(4 * N1CH, HWOUT)          # im2col streaming chunk

    assert n_dve + n_act <= 25

    nc = bacc.Bacc("TRN2", target_bir_lowering=False, debug=False)

    x_d = nc.dram_tensor("x", [B_PC, C, Hp, Wp], BF16, kind="ExternalInput").ap()
    im2col_d = nc.dram_tensor("im2col", [54, HWOUT], BF16, kind="ExternalInput").ap()
    wconv_d = nc.dram_tensor("wconv", [54, 128], BF16, kind="ExternalInput").ap()
    b1r_d = nc.dram_tensor("b1r", [128, 1], F32, kind="ExternalInput").ap()
    wft_d = nc.dram_tensor("wft", [65, 25, C], BF16, kind="ExternalInput").ap()
    ident_d = nc.dram_tensor("ident", [128, 128], BF16, kind="ExternalInput").ap()
    id32_d = nc.dram_tensor("id32", [128, 128], F32, kind="ExternalInput").ap()
    out_d = nc.dram_tensor("out", [B_PC, C, H, W], BF16, kind="ExternalOutput").ap()

    # dram scratch for layout bounces
    imgf_d = nc.dram_tensor("imgf_sc", [128], F32).ap()
    df_d = nc.dram_tensor("df_sc", [B_PC, 25, C], F32).ap()

    with tile.TileContext(nc) as tc:
        with (
            tc.tile_pool(name="consts", bufs=1) as consts,
            tc.tile_pool(name="p0", bufs=1) as p0,
            tc.tile_pool(name="imc", bufs=4) as imcp,
            tc.tile_pool(name="trash", bufs=2) as trashp,
            tc.tile_pool(name="xp", bufs=2) as xpp,
            # distinct diag tiles for all 4 slabs: a ring that aliases across
            # slabs would head-of-line-block the generating engine's queue
            tc.tile_pool(name="diag", bufs=NSLAB * 25) as diagp,
            tc.tile_pool(name="qt", bufs=max(n_dve // 2, 1) + 1) as qp,
            tc.tile_pool(name="at", bufs=max(n_act, 1) + 1) as atp,
            tc.tile_pool(name="et", bufs=3) as etp,
        ):
            # ---------- DMA: sync queue feeds conv1; ACT queue gets the rest
            wconv = consts.tile([54, 128], BF16)
            b1r = consts.tile([128, 1], F32)
            wft = consts.tile([65, 25, C], BF16)
            ident = consts.tile([128, 128], BF16)
            id32 = consts.tile([128, 128], F32)
            zeros = consts.tile([128, N1CH], F32)
            nc.sync.dma_start(wconv[:], wconv_d[:])
            nc.sync.dma_start(b1r[:], b1r_d[:])
            nc.scalar.dma_start(wft[:], wft_d[:])
            nc.scalar.dma_start(ident[:], ident_d[:])
            nc.scalar.dma_start(id32[:], id32_d[:])
            nc.vector.memset(zeros[:], 0.0)

            # whole im2col prefetched before the big x-slab loads so the
            # conv1 stream never starves behind them
            imts = []
            for o0 in range(0, HWOUT, IMCH):
                imt = imcp.tile([54, IMCH], BF16, tag="imc")
                nc.scalar.dma_start(imt[:], im2col_d[:, o0:o0 + IMCH])
                imts.append(imt)

            xps = [None] * NSLAB

            def load_slab(s):
                # flat alloc with 8 spare elements so full-width runs for the
                # bottom-most taps stay in-bounds
                b, cg = divmod(s, CG)
                xpf = xpp.tile([128, Hp * Wp + 8], BF16, tag="xp")
                nc.vector.memset(xpf[:, Hp * Wp:], 0.0)
                xp3 = xpf[:, 0:Hp * Wp].rearrange("p (a b) -> p a b", a=Hp, b=Wp)
                nc.scalar.dma_start(xp3, x_d[b, cg * 128:(cg + 1) * 128, :, :])
                xps[s] = (xpf, xp3)

            load_slab(0)
            load_slab(1)

            # ---------- head: conv1 + dense + softmax + transposes ----------
            acc = p0.tile([128, N1], F32)
            vts = []
            with tc.tile_pool(name="psA", bufs=4, space="PSUM") as psA:
                for ci in range(N1):
                    imt = imts[ci // (IMCH // N1CH)]
                    ps1 = psA.tile([128, N1CH], F32, tag="c")
                    off = (ci % (IMCH // N1CH)) * N1CH
                    nc.tensor.matmul(
                        ps1[:], wconv[:], imt[:, off:off + N1CH],
                        start=True, stop=True,
                    )
                    tr = trashp.tile([128, N1CH], BF16, tag="tr")
                    if ci % 2 == 0:
                        nc.scalar.activation(
                            tr[:], ps1[:], AF.Relu, bias=b1r[:], scale=1.0,
                            accum_out=acc[:, ci:ci + 1],
                        )
                    else:
                        nc.vector.scalar_tensor_tensor(
                            tr[:], ps1[:], b1r[:], zeros[:], ALU.add, ALU.max,
                            accum_out=acc[:, ci:ci + 1],
                        )
                sfeat = p0.tile([128, 1], F32)
                nc.vector.tensor_reduce(sfeat[:], acc[:], mybir.AxisListType.X, ALU.add)
                nc.sync.dma_start(imgf_d[:], sfeat[:])

                # img_feat (sums) transposed [64,2]; mean folded into the cast
                imgfT32 = p0.tile([65, B_PC], F32)
                nc.sync.dma_start(
                    imgfT32[0:64, :],
                    imgf_d[:].rearrange("(b o) -> o b", b=B_PC, o=64),
                )
                imgfT = p0.tile([65, B_PC], BF16)
                nc.vector.memset(imgfT[64:65, :], 1.0)
                nc.scalar.mul(imgfT[0:64, :], imgfT32[0:64, :], 1.0 / HWOUT)

                # dense: df[b, t, c] (+bias row); bounce PSUM->SBUF->DRAM
                t0 = 0
                di = 0
                while t0 < 25:
                    tw = min(2, 25 - t0)
                    psd = psA.tile([B_PC, 2, C], F32, tag="c")
                    nc.tensor.matmul(
                        psd[:, 0:tw, :], imgfT[:],
                        wft[:, t0:t0 + tw, :],
                        start=True, stop=True,
                    )
                    dfc = trashp.tile([B_PC, 2, C], F32, tag="dfc")
                    if di % 2 == 0:
                        nc.scalar.copy(dfc[:, 0:tw, :], psd[:, 0:tw, :])
                    else:
                        nc.vector.tensor_copy(dfc[:, 0:tw, :], psd[:, 0:tw, :])
                    nc.sync.dma_start(df_d[:, t0:t0 + tw, :], dfc[:, 0:tw, :])
                    t0 += tw
                    di += 1

                # softmax over channels; batch b parked at partition b*32 so
                # the PE transpose below sees base partitions in {0, 32}
                dfsb = p0.tile([B_PC * 32, C], F32)
                edf = p0.tile([B_PC * 32, C], F32)
                ssum = p0.tile([B_PC * 32, 1], F32)
                rsum = p0.tile([B_PC * 32, 1], F32)
                wsm = p0.tile([B_PC * 32, C], F32)
                for b in range(B_PC):
                    sl = slice(b * 32, b * 32 + 25)
                    nc.sync.dma_start(dfsb[sl, :], df_d[b])
                    nc.scalar.activation(edf[sl, :], dfsb[sl, :], AF.Exp)
                    nc.vector.tensor_reduce(
                        ssum[sl, :], edf[sl, :], mybir.AxisListType.X, ALU.add)
                    nc.vector.reciprocal(rsum[sl, :], ssum[sl, :])
                    nc.vector.tensor_scalar(
                        wsm[sl, :], edf[sl, :], rsum[sl, :], None, ALU.mult)

                # per-slab filter values [128(c), 25] via PE transpose
                for s in range(NSLAB):
                    b, cg = divmod(s, CG)
                    pst = psA.tile([128, 25], F32, tag="t")
                    nc.tensor.transpose(
                        pst[:], wsm[b * 32:b * 32 + 25, cg * 128:(cg + 1) * 128],
                        id32[b * 32:b * 32 + 25, 0:25],
                    )
                    vt = p0.tile([128, 25], F32, tag=f"vt{s}")
                    nc.scalar.copy(vt[:], pst[:])
                    vts.append(vt)

            # ---------- depthwise ----------
            # PE: n_pe taps (diag matmuls, PSUM accumulate); DVE: n_dve taps
            # as scalar_tensor_tensor chains accumulating straight into PSUM
            # (PSUM reads bypass the SBUF ports, so no shared-port traffic and
            # no fold passes); ACT: n_act taps as scale-multiplies folded into
            # the evacuated output by DVE tensor_tensor (one-quad lag).
            n_pe = 25 - n_dve - n_act
            pe_taps = TAPS[:n_pe]
            dve_taps = TAPS[n_pe:n_pe + n_dve]
            act_taps = TAPS[n_pe + n_dve:]

            dts_all = [None] * NSLAB
            _dgen_rr = [0]

            def gen_diags(s):
                # diag tiles for slab s, all on ScalarE (it is otherwise idle;
                # VectorE is the critical engine).  Only PE taps need diags.
                dts = [None] * 25
                for (i, j) in pe_taps:
                    t = i * K5 + j
                    dt_ = diagp.tile([128, 128], BF16, tag="dt")
                    nc.scalar.mul(dt_[:], ident[:], vts[s][:, t:t + 1])
                    dts[t] = dt_
                dts_all[s] = dts

            gen_diags(0)
            gen_diags(1)

            with tc.tile_pool(name="psQ", bufs=2, space="PSUM") as psQ:
                for s in range(NSLAB):
                    b, cg = divmod(s, CG)
                    vt = vts[s]
                    xpf, xp = xps[s]
                    dts = dts_all[s]
                    if s + 2 < NSLAB:
                        load_slab(s + 2)
                    if s + 1 < NSLAB and s > 0:
                        gen_diags(s + 1)

                    for q in range(NQ):
                        y0 = q * QR

                        def win(i, j):
                            return xp[:, y0 + i:y0 + i + QR, j:j + W]

                        def sc(i, j):
                            t = i * K5 + j
                            return vt[:, t:t + 1]

                        def run(i, j):
                            off = (y0 + i) * Wp + j
                            return xpf[:, off:off + QR * Wp]

                        # ACT taps -> partial tiles (optional)
                        ats = []
                        for (i, j) in act_taps:
                            at = atp.tile([128, QR * Wp], BF16, tag="at")
                            nc.scalar.mul(at[:], run(i, j), sc(i, j))
                            ats.append(at)

                        # DVE pair taps -> partial tiles (independent, so the
                        # DVE queue streams them back to back with no RAW gap)
                        qs = []
                        for pi in range(n_dve // 2):
                            ia, ja = dve_taps[2 * pi]
                            ib, jb = dve_taps[2 * pi + 1]
                            qt = qp.tile([128, QR * Wp], BF16, tag="q")
                            nc.vector._custom_dve(
                                PAIR_MAC_ANT, out=qt[:],
                                in0=run(ia, ja), in1=run(ib, jb),
                                s0=sc(ia, ja), s1=sc(ib, jb))
                            qs.append(qt)

                        # PE taps -> PSUM
                        ps = psQ.tile([128, QR, W], F32, tag="ps")
                        for g in range(QG):
                            gy = y0 + g * GR
                            for k, (i, j) in enumerate(pe_taps):
                                nc.tensor.matmul(
                                    ps[:, g * GR:(g + 1) * GR, :],
                                    dts[i * K5 + j][:],
                                    xp[:, gy + i:gy + i + GR, j:j + W],
                                    start=(k == 0),
                                    stop=(k == len(pe_taps) - 1),
                                )

                        def v(tile_):
                            return tile_[:].rearrange(
                                "p (a b) -> p a b", a=QR, b=Wp)[:, :, 0:W]

                        # fold pair partials (stock TT runs at 2x) and ACT
                        # partials into qs[0]
                        for k in range(1, len(qs)):
                            nc.vector.tensor_tensor(
                                qs[0][:], qs[0][:], qs[k][:], ALU.add)
                        for at in ats:
                            nc.vector.tensor_tensor(
                                qs[0][:], qs[0][:], at[:], ALU.add)
                        if n_dve % 2:
                            i, j = dve_taps[-1]
                            nc.vector.scalar_tensor_tensor(
                                ps[:], win(i, j), sc(i, j), ps[:],
                                ALU.mult, ALU.add)

                        # ACT evacuates PSUM the moment PE stops (DVE never
                        # touches ps with pure pair taps, so PSUM turnover is
                        # decoupled from the DVE stream); DVE folds its side in
                        # with one 2x tensor_tensor.
                        et = etp.tile([128, QR, W], BF16, tag="et")
                        nc.scalar.copy(et[:], ps[:])
                        if qs:
                            nc.vector.tensor_tensor(
                                et[:], et[:], v(qs[0]), ALU.add)
                        nc.sync.dma_start(
                            out_d[b, cg * 128:(cg + 1) * 128, y0:y0 + QR, :],
                            et[:])

    nc.compile()
    return nc


def get_program(H, W, n_dve=8, n_act=0):
    key = (H, W, n_dve, n_act)
    if key not in _PROG_CACHE:
        _PROG_CACHE[key] = _build_program(H, W, n_dve, n_act)
    return _PROG_CACHE[key]


def host_prep(x_feat, raw_img, w_conv1, b_conv1, g1, beta1, m1, v1,
              w_filt, b_filt, g2, beta2, m2, v2):
    """Fold BN params, build im2col + packed weights; returns per-core in_maps."""
    B, Cc, H, W = x_feat.shape
    assert Cc == C
    n_cores = B // B_PC

    a1 = g1 / np.sqrt(v1 + EPS)
    w1f = (w_conv1 * a1[:, None, None, None]).astype(np.float32)   # (64,3,3,3)
    b1f = (b_conv1 - m1) * a1 + beta1                               # (64,)

    a2 = g2 / np.sqrt(v2 + EPS)
    wff = (w_filt * a2[:, None]).astype(np.float32)                 # (6400,64)
    bff = (b_filt - m2) * a2 + beta2                                # (6400,)

    # wft[k, t, c]: k<64 -> wff[c*25+t, k]; k=64 -> bias row (contiguous
    # channel runs so the dense matmul's moving operand streams fast)
    wft = np.empty((65, 25, C), np.float32)
    wft[:64] = wff.reshape(C, 25, 64).transpose(2, 1, 0)
    wft[64] = bff.reshape(C, 25).T
    wft16 = wft.astype(ml_dtypes.bfloat16)

    b1r = np.tile(b1f, B_PC).reshape(128, 1).astype(np.float32)

    ident = np.eye(128, dtype=ml_dtypes.bfloat16)
    # 25x25 identity blocks at partition offsets 0 and 32 (PE-transpose
    # requires the identity operand at the same base partition as the input)
    id32 = np.zeros((128, 128), np.float32)
    for b in range(B_PC):
        id32[b * 32:b * 32 + 25, 0:25] = np.eye(25)

    xpad16 = np.pad(x_feat, ((0, 0), (0, 0), (2, 2), (2, 2)),
                    mode="reflect").astype(ml_dtypes.bfloat16)

    # conv1 im2col, zero pad 1: [54, H*W] per core
    rawpad = np.pad(raw_img, ((0, 0), (0, 0), (1, 1), (1, 1))).astype(np.float32)

    # wconv[b*27 + (c*9+i*3+j), b*64+o] = w1f[o, c, i, j]
    wconv = np.zeros((54, 128), np.float32)
    w_flat = w1f.transpose(1, 2, 3, 0).reshape(27, 64)  # (c*9+i*3+j, o)
    for b in range(B_PC):
        wconv[b * 27:(b + 1) * 27, b * 64:(b + 1) * 64] = w_flat
    wconv16 = wconv.astype(ml_dtypes.bfloat16)

    in_maps = []
    for core in range(n_cores):
        bs = core * B_PC
        im2col = np.empty((54, H * W), np.float32)
        for b in range(B_PC):
            for c in range(3):
                for i in range(3):
                    for j in range(3):
                        p = b * 27 + c * 9 + i * 3 + j
                        im2col[p] = rawpad[bs + b, c, i:i + H, j:j + W].reshape(-1)
        in_maps.append({
            "x": xpad16[bs:bs + B_PC],
            "im2col": im2col.astype(ml_dtypes.bfloat16),
            "wconv": wconv16,
            "b1r": b1r,
            "wft": wft16,
            "ident": ident,
            "id32": id32,
        })
    return in_maps


def run(inputs, trace=False, n_dve=8, n_act=0):
    x_feat = inputs["x_feat"]
    B, _, H, W = x_feat.shape
    nc = get_program(H, W, n_dve, n_act)
    in_maps = host_prep(**inputs)
    n_cores = len(in_maps)
    res = run_bass_kernel_spmd(nc, in_maps, list(range(n_cores)), trace=trace)
    out = np.concatenate(
        [r["out"].astype(np.float32) for r in res.results], axis=0)
    return out, res


def kernel(**inputs) -> np.ndarray:
    out, _ = run(inputs, trace=False)
    return out
